# revision 29
# baseline (speedup 1.0000x reference)
"""Bass/Tile kernel for multi-head self-attention on 8 TRN2 NeuronCores.

Problem: B=16, S=1024, D=768, H=12, head_dim=64, fp32 in/out.
Strategy: data parallel over batch (2 batch items per core, no collectives).

Layout follows the proven baseline (bf16 matmul operands, fp32 accum):
  - xT [c, t] feature-major via PE transposes; weights pre-transposed to
    wT [c_in, c_out]; qT/kT feature-major [o, t]; v token-major with a
    ones column per head (v_aug) so P@V emits softmax denominators in
    its last output row; scoresT = kT_h.T @ qT_h with exp on ScalarE
    straight out of PSUM (2 banks, FD=1024, +ln16 bias).

Scheduling is rebuilt around one principle: the PE queue is in-order,
so every instruction that waits on the exp stream (QK needing a free
score bank, PV needing probsT) must have independent filler matmuls
emitted BEFORE it, or the PE idles and drops out of its p-state ramp
(2.4 -> 1.2 GHz).  Concretely:
  - QK/PV emission is interleaved with filler units pulled from a
    per-pair work list: next q/k projection chunk, v sweeps (split in
    three 4-head sweeps), next batch's x transposes, wo transpose,
    and the previous batch's output projection.
  - Normalization is per-pair: P@V's sum row is DMA'd straight from
    PSUM into a [2, S] staging tile, reciprocal'd on DVE, and the
    selector-matmul broadcast + multiply run one pair later while the
    PE is warm.  The selector is a single [2, 128] constant (head 0 of
    the pair -> rows 0-63, head 1 -> rows 64-127 of its chunk).
  - The output projection is split: cc 0-3 accumulate into bf16
    out_bf during pairs 4-5 (norm(3) is done by then); cc 4-5 + bias
    + DMA run as soon as norm(5) lands (a short tail for the last
    batch, filler inside the next batch's pair 0 otherwise).
  - ScalarE runs nothing but exp during the pair loops (weight casts
    and broadcast evacuations live on DVE), so the exp stream is
    never stalled by its own engine.
"""

import contextlib
import threading
from collections import deque

import numpy as np

import concourse.bass as bass
import concourse.tile as tile
from concourse import bacc, mybir
from concourse.bass_utils import run_bass_kernel_spmd
from concourse.masks import make_identity

N_CORES = 8
B, S, D = 16, 1024, 768
H, HD = 12, 64
BPC = B // N_CORES  # batch items per core

P = 128
CC = D // P          # 6 feature chunks of 128
TN = 512             # matmul moving free dim
NT = S // TN         # 2 token chunks of 512
TT = S // P          # 8 token tiles of 128
JT = S // P          # 8 key tiles of 128
HPC = P // HD        # 2 heads per feature chunk
NP = H // 2          # 6 head pairs

F32 = mybir.dt.float32
BF16 = mybir.dt.bfloat16
F32R = mybir.dt.float32r

AF = mybir.ActivationFunctionType
ALU = mybir.AluOpType


class Filler:
    """FIFO of small emission closures interleaved into the PE stream."""

    def __init__(self):
        self.q = deque()

    def add(self, units):
        self.q.extend(units)

    def run(self, n=1):
        for _ in range(n):
            if self.q:
                self.q.popleft()()

    def drain(self):
        while self.q:
            self.q.popleft()()


def build_kernel(tc: "tile.TileContext", outs, ins):
    nc = tc.nc
    x_d = ins["x"]
    out_d = outs["out"]

    ctx = contextlib.ExitStack()
    with ctx:
        const = ctx.enter_context(tc.tile_pool(name="const", bufs=1))
        wpool = ctx.enter_context(tc.tile_pool(name="wts", bufs=1))
        iop = ctx.enter_context(tc.tile_pool(name="iop", bufs=3))
        work = ctx.enter_context(tc.tile_pool(name="work", bufs=1))
        probs_pool = ctx.enter_context(tc.tile_pool(name="probs", bufs=1))
        small = ctx.enter_context(tc.tile_pool(name="small", bufs=2))
        # PSUM budget (8 banks): sq 2x2 + pv 2x1 + mm 2x1 = 8
        psum = ctx.enter_context(tc.tile_pool(name="psum", bufs=1, space="PSUM"))

        # ---- one-time constants ----
        ones_row = const.tile([1, P], F32)
        nc.vector.memset(ones_row, 1.0)
        ones_r = const.tile([1, P], F32R)
        nc.vector.tensor_copy(ones_r, ones_row)

        # +ln(16) folded into exp keeps probs away from the denormal range;
        # the factor cancels between P@V numerator and denominators
        ln16 = const.tile([P, 1], F32)
        nc.vector.memset(ln16, float(np.log(16.0)))

        # q/k biases laid out per-partition: [p, oc] = b[oc*128 + p]
        bq = const.tile([P, CC], F32)
        bk = const.tile([P, CC], F32)
        with nc.allow_non_contiguous_dma(reason="tiny bias load"):
            nc.sync.dma_start(bq, ins["wq_b"].rearrange("(oc p) -> p oc", p=P))
            nc.sync.dma_start(bk, ins["wk_b"].rearrange("(oc p) -> p oc", p=P))

        # ---- x / weight streaming ----
        def x_dma(b, tt):
            xf = iop.tile([P, D], F32, tag="nat_f", name="xf", bufs=4)
            nc.sync.dma_start(xf, x_d[b, tt * P : (tt + 1) * P, :])
            return xf

        xT = work.tile([P, CC, S], BF16, tag="xT", name="xT")

        def _cast(dst, src, eng):
            if eng == "scalar":
                nc.scalar.copy(dst, src)
            else:
                nc.vector.tensor_copy(dst, src)

        def xT_unit(xf, tt, ptr_tag=None, cast_eng="vector"):
            # cast to bf16, then XBAR DMA-transpose straight into the
            # feature-major layout: out[p, c, q] = xb[q, c*128+p].
            # No PE or PSUM involvement at all.
            def emit():
                xb = iop.tile([P, D], BF16, tag="nat_b", name="xb", bufs=4)
                _cast(xb, xf, cast_eng)
                nc.sync.dma_start_transpose(
                    xT[:, :, tt * P : (tt + 1) * P], xb
                )

            return emit

        wT = {}
        for wname in ("wq_w", "wk_w", "wv_w", "wo_w"):
            wT[wname] = wpool.tile([P, CC, D], BF16, name=f"{wname}_T")

        def w_dma(name, oc, eng=None):
            wf = iop.tile([P, D], F32, tag="nat_f", name="wf", bufs=4)
            (eng or nc.gpsimd).dma_start(wf, ins[name][oc * P : (oc + 1) * P, :])
            return wf

        def wT_unit(name, wf, oc, ptr_tag=None, cast_eng="vector"):
            def emit():
                wb = iop.tile([P, D], BF16, tag="nat_b", name="wb", bufs=4)
                _cast(wb, wf, cast_eng)
                nc.sync.dma_start_transpose(
                    wT[name][:, :, oc * P : (oc + 1) * P], wb
                )

            return emit

        # v/out biases broadcast along partitions: [128, 768] via ones-matmul
        bias_bc = {}

        def build_bias_bc():
            for name in ("wv_b", "wo_b"):
                brow_f = iop.tile([1, D], F32, name=f"{name}_rowf", tag="nat_f", bufs=4)
                nc.sync.dma_start(brow_f, ins[name][None, :])
                brow = iop.tile([1, D], F32R, name=f"{name}_row", tag="brow", bufs=1)
                nc.vector.tensor_copy(brow, brow_f)
                bc = const.tile([P, D], BF16, name=f"{name}_bc")
                for n0 in range(0, D, TN):
                    nsz = min(TN, D - n0)
                    pb = psum.tile([P, TN], F32, tag="mm", bufs=2, name="pbias")
                    nc.tensor.matmul(
                        pb[:, :nsz], ones_r, brow[:, n0 : n0 + nsz],
                        start=True, stop=True,
                    )
                    nc.vector.tensor_copy(bc[:, n0 : n0 + nsz], pb[:, :nsz])
                bias_bc[name] = bc

        # ---- per-batch persistent tiles ----
        qT = work.tile([P, CC, S], BF16, tag="qT", name="qT")
        kT = work.tile([P, CC, S], BF16, tag="kT", name="kT")
        v_aug = work.tile([P, TT, H, HD + 1], BF16, tag="v_aug", name="v_aug")
        nc.vector.memset(v_aug[:, :, :, HD : HD + 1], 1.0)
        out_bf = work.tile([P, TT, D], BF16, tag="out_bf", name="out_bf")

        def proj_qk_unit(dst, wname, bap, oc, nt):
            def emit():
                wt = wT[wname]
                pq = psum.tile([P, TN], F32, tag="mm", bufs=2, name="pq")
                for cc in range(CC):
                    nc.tensor.matmul(
                        pq,
                        wt[:, cc, oc * P : (oc + 1) * P],
                        xT[:, cc, nt * TN : (nt + 1) * TN],
                        start=(cc == 0),
                        stop=(cc == CC - 1),
                    )
                nc.vector.tensor_tensor(
                    dst[:, oc, nt * TN : (nt + 1) * TN],
                    pq,
                    bap[:, oc : oc + 1].to_broadcast((P, TN)),
                    ALU.add,
                )

            return emit

        def chunk_units(oc):
            return [
                proj_qk_unit(dst, wname, bap, oc, nt)
                for dst, wname, bap in ((qT, "wq_w", bq), (kT, "wk_w", bk))
                for nt in range(NT)
            ]

        QTN = 256  # v sweep width: 4 heads per sweep

        def v_unit(sw, mt):
            def emit():
                n0 = sw * QTN
                wv = wT["wv_w"]
                pv = psum.tile([P, TN], F32, tag="mm", bufs=2, name="pv")
                for cc in range(CC):
                    nc.tensor.matmul(
                        pv[:, :QTN],
                        xT[:, cc, mt * P : (mt + 1) * P],
                        wv[:, cc, n0 : n0 + QTN],
                        start=(cc == 0),
                        stop=(cc == CC - 1),
                    )
                h0 = n0 // HD
                nh = QTN // HD
                nc.vector.tensor_tensor(
                    v_aug[:, mt, h0 : h0 + nh, 0:HD],
                    pv[:, :QTN].rearrange("p (h d) -> p h d", d=HD),
                    bias_bc["wv_b"][:, n0 : n0 + QTN].rearrange(
                        "p (h d) -> p h d", d=HD
                    ),
                    ALU.add,
                )

            return emit

        def v_units(sw):
            return [v_unit(sw, mt) for mt in range(TT)]

        # ---- output projection, split cc 0-3 (A) and cc 4-5 (B) ----
        def phaseA_unit(attn_T, mt, n0, nsz):
            def emit():
                wo = wT["wo_w"]
                pf = psum.tile([P, TN], F32, tag="mm", bufs=2, name="pfA")
                for cc in range(4):
                    nc.tensor.matmul(
                        pf[:, :nsz],
                        attn_T[:, cc, mt * P : (mt + 1) * P],
                        wo[:, cc, n0 : n0 + nsz],
                        start=(cc == 0),
                        stop=(cc == 3),
                    )
                nc.vector.tensor_tensor(
                    out_bf[:, mt, n0 : n0 + nsz],
                    pf[:, :nsz],
                    bias_bc["wo_b"][:, n0 : n0 + nsz],
                    ALU.add,
                )

            return emit

        def phaseA_units(attn_T):
            return [
                phaseA_unit(attn_T, mt, n0, min(TN, D - n0))
                for mt in range(TT)
                for n0 in range(0, D, TN)
            ]

        def phaseB_unit(b, attn_T, mt):
            def emit():
                wo = wT["wo_w"]
                out_sb = iop.tile([P, D], F32, tag="out_sb", bufs=2, name="out_sb")
                for n0 in range(0, D, TN):
                    nsz = min(TN, D - n0)
                    pf = psum.tile([P, TN], F32, tag="mm", bufs=2, name="pfB")
                    for cc in (4, 5):
                        nc.tensor.matmul(
                            pf[:, :nsz],
                            attn_T[:, cc, mt * P : (mt + 1) * P],
                            wo[:, cc, n0 : n0 + nsz],
                            start=(cc == 4),
                            stop=(cc == 5),
                        )
                    nc.vector.tensor_tensor(
                        out_sb[:, n0 : n0 + nsz],
                        pf[:, :nsz],
                        out_bf[:, mt, n0 : n0 + nsz],
                        ALU.add,
                    )
                nc.sync.dma_start(out_d[b, mt * P : (mt + 1) * P, :], out_sb)

            return emit

        def phaseB_units(b, attn_T):
            return [phaseB_unit(b, attn_T, mt) for mt in range(TT)]

        def out_full_unit(b, attn_T, mt):
            # single-pass output projection (all 6 chunks); used for b0
            # whose whole out_proj runs as filler inside b1's pairs
            def emit():
                wo = wT["wo_w"]
                out_sb = iop.tile([P, D], F32, tag="out_sb", bufs=2, name="out_sbf")
                for n0 in range(0, D, TN):
                    nsz = min(TN, D - n0)
                    pf = psum.tile([P, TN], F32, tag="mm", bufs=2, name="pff")
                    for cc in range(CC):
                        nc.tensor.matmul(
                            pf[:, :nsz],
                            attn_T[:, cc, mt * P : (mt + 1) * P],
                            wo[:, cc, n0 : n0 + nsz],
                            start=(cc == 0),
                            stop=(cc == CC - 1),
                        )
                    nc.vector.tensor_tensor(
                        out_sb[:, n0 : n0 + nsz],
                        pf[:, :nsz],
                        bias_bc["wo_b"][:, n0 : n0 + nsz],
                        ALU.add,
                    )
                nc.sync.dma_start(out_d[b, mt * P : (mt + 1) * P, :], out_sb)

            return emit

        # ---- per-pair normalization (per token-half ic), PE-free:
        # recip rows live in two single-row tiles (partition 0 each),
        # GpSimd broadcasts them across the head's 64 partitions, DVE
        # multiplies attn in place.
        def norm_ic(attn_T, pair, rsAB, ic):
            # broadcast [1, 2, TN] (partition 0) to all 128 partitions,
            # then per-half multiplies pick the right recip row
            icsl = slice(ic * TN, (ic + 1) * TN)
            rb = small.tile([P, 2, TN], F32, tag="rb", bufs=2)
            nc.gpsimd.partition_broadcast(rb, rsAB[0:1, :, icsl])
            sl = attn_T[:, pair, icsl]
            nc.vector.tensor_tensor(
                sl[0:HD, :], sl[0:HD, :], rb[0:HD, 0, :], ALU.mult
            )
            nc.vector.tensor_tensor(
                sl[HD:P, :], sl[HD:P, :], rb[HD:P, 1, :], ALU.mult
            )

        def make_norm(attn_T, pair, rsAB):
            def emit():
                for ic in range(NT):
                    norm_ic(attn_T, pair, rsAB, ic)

            return emit

        # ---- attention pair ----
        # Software-pipelined: QK(k) is paced by its score bank (freed by
        # EXP k-2); PV steps for EXP k-2 ride in the same slot, so the
        # PE stream self-paces with the exp stream with ~0.2us/slot to
        # spare.  External fillers (fs=small ~0.6us, fb=big ~1.4us) slot
        # into the ic1 drain phase and every few triples.
        def emit_pair(attn_T, pair, fs, fb, pre=None, post_ic0=None):
            # pre (prev pair's normalization) is PE-free; run it before
            # this pair re-allocates the single-buffered rsAB/stage ring
            # slots so its reads precede the new generation
            if pre is not None:
                pre()
            probsT_ic = []
            pos = {}
            rsAB = small.tile([1, 2, S], F32, tag="rsAB", bufs=1)
            stage = small.tile([P, 2, TN], F32, tag="stage", bufs=1)

            def qk_step(s):
                ic, jt = divmod(s, JT)
                if jt == 0:
                    probsT = probs_pool.tile(
                        [P, 2, JT, TN], BF16, tag="probsT", bufs=2,
                        name="probsT",
                    )
                    probsT_ic.append(probsT)
                sq = psum.tile([P, 2, TN], F32, tag="sq", bufs=2, name="sq")
                for hi in range(2):
                    hp = hi * HD
                    nc.tensor.matmul(
                        sq[:, hi],
                        kT[hp : hp + HD, pair, jt * P : (jt + 1) * P],
                        qT[hp : hp + HD, pair, ic * TN : (ic + 1) * TN],
                        start=True,
                        stop=True,
                    )
                nc.scalar.activation(
                    probsT_ic[ic][:, :, jt, :],
                    sq,
                    AF.Exp,
                    bias=ln16,
                    scale=float(1.0 / np.sqrt(HD)),
                )

            def pv_step(ic, jt, hi):
                h = pair * 2 + hi
                if (ic, hi) not in pos:
                    pos[(ic, hi)] = psum.tile(
                        [P, TN], F32, tag="pv", bufs=2, name="po"
                    )
                nc.tensor.matmul(
                    pos[(ic, hi)][: HD + 1, :],
                    v_aug[:, jt, h, :],
                    probsT_ic[ic][:, hi, jt, :],
                    start=(jt == 0),
                    stop=(jt == JT - 1),
                )

            def pv_evac(ic, hi):
                icsl = slice(ic * TN, (ic + 1) * TN)
                po = pos[(ic, hi)]
                nc.vector.tensor_copy(
                    stage[HD : HD + 1, hi, :], po[HD : HD + 1, :]
                )
                if hi == 0:
                    nc.vector.tensor_copy(attn_T[0:HD, pair, icsl], po[:HD, :])
                else:
                    # DVE lanes can't cross partitions; bounce via DMA
                    tmp = small.tile([HD, TN], BF16, tag="odd_tmp", bufs=2)
                    nc.vector.tensor_copy(tmp, po[:HD, :])
                    nc.gpsimd.dma_start(attn_T[HD:P, pair, icsl], tmp)

            def recip_ic(ic):
                # NOTE: reciprocal_approx_fast requires base partition 0,
                # so the sums hop to rsAB first and reciprocal in place
                icsl = slice(ic * TN, (ic + 1) * TN)
                nc.gpsimd.dma_start(
                    rsAB[0:1, :, icsl], stage[HD : HD + 1, :, :]
                )
                nc.vector.reciprocal_approx_fast(
                    rsAB[0:1, :, icsl], rsAB[0:1, :, icsl]
                )

            # ic0 phase: QK s0..15 with PV(ic0) steps two EXPs behind
            qk_step(0)
            qk_step(1)
            for s in range(2, 2 * JT):
                qk_step(s)
                e = s - 2
                jt, hi = divmod(e, 2)
                pv_step(0, jt, hi)
                if s % 3 == 1:
                    fs.run(1)
            # drain phase: last PV(ic0) steps, evac, recip; then PV(ic1)
            pv_step(0, 7, 0)
            pv_step(0, 7, 1)
            pv_evac(0, 0)
            pv_evac(0, 1)
            recip_ic(0)
            if post_ic0 is not None:
                post_ic0(rsAB)
            for e in range(2 * JT):
                jt, hi = divmod(e, 2)
                pv_step(1, jt, hi)
                if e % 2 == 0:
                    (fb if e % 4 == 0 else fs).run(1)
            pv_evac(1, 0)
            pv_evac(1, 1)
            recip_ic(1)
            return rsAB

        # ================= emission schedule (BPC == 2) =================
        # Startup: x(b0) + first weight chunks; get to pair 0 fast.
        # DMA issue order on sync: x0-3, wq0, wk0, x4-7 (so the first
        # weight chunks arrive early enough to transpose between x tiles).
        wq0 = w_dma("wq_w", 0)
        wk0 = w_dma("wk_w", 0)
        xfs = [x_dma(0, tt) for tt in range(TT)]
        wvf = [w_dma("wv_w", oc) for oc in range(CC)]
        build_bias_bc()
        for tt in range(TT):
            xT_unit(xfs[tt], tt, cast_eng="scalar")()
        wT_unit("wq_w", wq0, 0, cast_eng="scalar")()
        wT_unit("wk_w", wk0, 0, cast_eng="scalar")()
        for u in chunk_units(0):
            u()
        for oc in range(CC):
            wT_unit("wv_w", wvf[oc], oc, cast_eng="scalar")()
        for u in v_units(0):
            u()
        # remaining weight DMAs (sync queue, in consumption order)
        wqk_rest = [
            (name, oc, w_dma(name, oc, eng=nc.sync))
            for oc in range(1, CC)
            for name in ("wq_w", "wk_w")
        ]
        wof = [w_dma("wo_w", oc, eng=nc.sync) for oc in range(CC)]
        wqkT = [wT_unit(n, f, oc) for n, oc, f in wqk_rest]
        woT = [wT_unit("wo_w", wof[oc], oc) for oc in range(CC)]

        # ---- batch 0 ----
        attn0 = work.tile([P, CC, S], BF16, tag="attn_T", bufs=2, name="attn0")
        fs0 = [Filler() for _ in range(NP)]
        fb0 = [Filler() for _ in range(NP)]
        fs0[0].add(wqkT[0:4] + v_units(1)[:4])
        fb0[0].add(chunk_units(1))
        fs0[1].add(wqkT[4:10] + v_units(1)[4:])
        fb0[1].add(chunk_units(2))
        fs0[2].add(woT + v_units(2)[:4])
        fb0[2].add(chunk_units(3))
        fs0[3].add(v_units(2)[4:])
        fb0[3].add(chunk_units(4))
        fb0[4].add(chunk_units(5))
        recips0 = []
        for pair in range(NP):
            if pair == 3:
                # next batch x: DMA now (sync queue, weights all sent);
                # transpose during pairs 4-5 once chunk5/v-sweep reads of
                # this batch's xT are emitted
                # all x(b1) transposes go to pair 5's slots: they
                # overwrite xT, which chunk5 units (fb0[4]) still read
                nxfs = [x_dma(1, tt) for tt in range(TT)]
                xTb1 = [xT_unit(nxfs[tt], tt) for tt in range(TT)]
                fs0[5].add(xTb1)
            pre = (
                make_norm(attn0, pair - 1, recips0[pair - 1])
                if pair > 0
                else None
            )
            recips0.append(
                emit_pair(attn0, pair, fs0[pair], fb0[pair], pre=pre)
            )
            fb0[pair].drain()
            fs0[pair].drain()

        # batch 1 preamble: its chunk 0 projection and the first half of
        # its v sweep 0 must precede its pair 0's QK/PV emissions; b0's
        # last norm runs after them so its recip has resolved
        vb1 = v_units(0)
        for u in chunk_units(0):
            u()
        for u in vb1[:4]:
            u()
        make_norm(attn0, NP - 1, recips0[NP - 1])()

        # ---- batch 1 ----
        attn1 = work.tile([P, CC, S], BF16, tag="attn_T", bufs=2, name="attn1")
        pa1 = phaseA_units(attn1)
        pb1 = phaseB_units(1, attn1)
        out0 = [out_full_unit(0, attn0, mt) for mt in range(TT)]
        fs1 = [Filler() for _ in range(NP)]
        fb1 = [Filler() for _ in range(NP)]
        fs1[0].add(vb1[4:])
        fb1[0].add(chunk_units(1))
        fs1[1].add(v_units(1))
        fb1[1].add(chunk_units(2) + out0[:2])
        fb1[2].add(chunk_units(3) + out0[2:5])
        fs1[3].add(v_units(2))
        fb1[3].add(chunk_units(4) + out0[5:7])
        fs1[4].add(pa1[:6])
        fb1[4].add(chunk_units(5) + out0[7:])
        fs1[5].add(pa1[6:])

        def tail_post(recip2, fs=None, fb=None):
            # after pair 5's ic0 recip: drain remaining phase-A fillers,
            # normalize chunk 5's first token half, and project+store
            # token tiles 0-3 while ic1's exp stream is still going
            fs.drain()
            fb.drain()
            norm_ic(attn1, NP - 1, recip2, 0)
            for u in pb1[:4]:
                u()

        import functools

        recips1 = []
        for pair in range(NP):
            pre = (
                make_norm(attn1, pair - 1, recips1[pair - 1])
                if pair > 0
                else None
            )
            post = (
                functools.partial(tail_post, fs=fs1[pair], fb=fb1[pair])
                if pair == NP - 1
                else None
            )
            recips1.append(
                emit_pair(attn1, pair, fs1[pair], fb1[pair], pre=pre,
                          post_ic0=post)
            )
            fb1[pair].drain()
            fs1[pair].drain()
        norm_ic(attn1, NP - 1, recips1[NP - 1], 1)
        for u in pb1[4:]:
            u()


_BUILD_LOCK = threading.Lock()
_BUILT = {}


def build():
    with _BUILD_LOCK:
        if "nc" in _BUILT:
            return _BUILT["nc"]
        nc = bacc.Bacc(
            "TRN2",
            target_bir_lowering=False,
            debug=False,
            enable_asserts=True,
            num_devices=N_CORES,
        )
        ins = {
            "x": nc.dram_tensor("x", [BPC, S, D], F32, kind="ExternalInput").ap(),
        }
        for w in ("wq_w", "wk_w", "wv_w", "wo_w"):
            ins[w] = nc.dram_tensor(w, [D, D], F32, kind="ExternalInput").ap()
        for bn in ("wq_b", "wk_b", "wv_b", "wo_b"):
            ins[bn] = nc.dram_tensor(bn, [D], F32, kind="ExternalInput").ap()
        outs = {
            "out": nc.dram_tensor(
                "out", [BPC, S, D], F32, kind="ExternalOutput"
            ).ap()
        }
        with tile.TileContext(nc) as tc:
            build_kernel(tc, outs, ins)
        nc.compile()
        _BUILT["nc"] = nc
        return nc


def make_in_maps(inputs):
    x = np.ascontiguousarray(np.asarray(inputs["x"], dtype=np.float32))
    shared = {
        k: np.ascontiguousarray(np.asarray(inputs[k], dtype=np.float32))
        for k in (
            "wq_w", "wq_b", "wk_w", "wk_b", "wv_w", "wv_b", "wo_w", "wo_b",
        )
    }
    in_maps = []
    for c in range(N_CORES):
        m = {"x": x[c * BPC : (c + 1) * BPC]}
        m.update(shared)
        in_maps.append(m)
    return in_maps


def _ensure_profile_hook():
    """Install the axon NTFF profile hook shim if the container lacks it."""
    try:
        from antenv.axon_hooks import get_axon_ntff_profile_hook  # noqa: F401

        return
    except ImportError:
        pass
    try:
        import sys
        import types

        from trn_agent_boot.trn_boot import _ntff_profile_via_ctypes

        state = {"h": None}
        mod = types.ModuleType("antenv.axon_hooks")
        mod.set_axon_ntff_profile_hook = lambda h: state.__setitem__("h", h)
        mod.get_axon_ntff_profile_hook = lambda: state["h"]
        sys.modules["antenv.axon_hooks"] = mod
        mod.set_axon_ntff_profile_hook(
            _ntff_profile_via_ctypes("/opt/axon/libaxon_pjrt.so")
        )

        import concourse.bass_utils as bu

        orig_upload = bu.upload_artifacts

        def _safe_upload(d, *a, **k):
            try:
                return orig_upload(d, *a, **k)
            except Exception:
                return str(d)

        bu.upload_artifacts = _safe_upload
    except Exception:
        pass


def run(inputs, trace=False, **kwargs):
    """Returns (full_output [B,S,D] f32, BassKernelResults)."""
    if trace:
        _ensure_profile_hook()
    nc = build()
    res = run_bass_kernel_spmd(
        nc, make_in_maps(inputs), core_ids=list(range(N_CORES)),
        trace=trace, **kwargs,
    )
    out = np.concatenate([res.results[c]["out"] for c in range(N_CORES)], axis=0)
    return out, res


def kernel(**inputs):
    try:
        out, _ = run(inputs, trace=False)
    except Exception:
        # transient device hiccups (e.g. a prior crashed session) recover
        # on retry; the graph is already built/compiled at this point
        out, _ = run(inputs, trace=False)
    return out


# revision 30
# speedup vs baseline: 1.0242x; 1.0242x over previous
"""Bass/Tile kernel for multi-head self-attention on 8 TRN2 NeuronCores.

Problem: B=16, S=1024, D=768, H=12, head_dim=64, fp32 in/out.
Strategy: data parallel over batch (2 batch items per core, no collectives).

Layout follows the proven baseline (bf16 matmul operands, fp32 accum):
  - xT [c, t] feature-major via PE transposes; weights pre-transposed to
    wT [c_in, c_out]; qT/kT feature-major [o, t]; v token-major with a
    ones column per head (v_aug) so P@V emits softmax denominators in
    its last output row; scoresT = kT_h.T @ qT_h with exp on ScalarE
    straight out of PSUM (2 banks, FD=1024, +ln16 bias).

Scheduling is rebuilt around one principle: the PE queue is in-order,
so every instruction that waits on the exp stream (QK needing a free
score bank, PV needing probsT) must have independent filler matmuls
emitted BEFORE it, or the PE idles and drops out of its p-state ramp
(2.4 -> 1.2 GHz).  Concretely:
  - QK/PV emission is interleaved with filler units pulled from a
    per-pair work list: next q/k projection chunk, v sweeps (split in
    three 4-head sweeps), next batch's x transposes, wo transpose,
    and the previous batch's output projection.
  - Normalization is per-pair: P@V's sum row is DMA'd straight from
    PSUM into a [2, S] staging tile, reciprocal'd on DVE, and the
    selector-matmul broadcast + multiply run one pair later while the
    PE is warm.  The selector is a single [2, 128] constant (head 0 of
    the pair -> rows 0-63, head 1 -> rows 64-127 of its chunk).
  - The output projection is split: cc 0-3 accumulate into bf16
    out_bf during pairs 4-5 (norm(3) is done by then); cc 4-5 + bias
    + DMA run as soon as norm(5) lands (a short tail for the last
    batch, filler inside the next batch's pair 0 otherwise).
  - ScalarE runs nothing but exp during the pair loops (weight casts
    and broadcast evacuations live on DVE), so the exp stream is
    never stalled by its own engine.
"""

import contextlib
import threading
from collections import deque

import numpy as np

import concourse.bass as bass
import concourse.tile as tile
from concourse import bacc, mybir
from concourse.bass_utils import run_bass_kernel_spmd
from concourse.masks import make_identity

N_CORES = 8
B, S, D = 16, 1024, 768
H, HD = 12, 64
BPC = B // N_CORES  # batch items per core

P = 128
CC = D // P          # 6 feature chunks of 128
TN = 512             # matmul moving free dim
NT = S // TN         # 2 token chunks of 512
TT = S // P          # 8 token tiles of 128
JT = S // P          # 8 key tiles of 128
HPC = P // HD        # 2 heads per feature chunk
NP = H // 2          # 6 head pairs

F32 = mybir.dt.float32
BF16 = mybir.dt.bfloat16
F32R = mybir.dt.float32r

AF = mybir.ActivationFunctionType
ALU = mybir.AluOpType


class Filler:
    """FIFO of small emission closures interleaved into the PE stream."""

    def __init__(self):
        self.q = deque()

    def add(self, units):
        self.q.extend(units)

    def run(self, n=1):
        for _ in range(n):
            if self.q:
                self.q.popleft()()

    def drain(self):
        while self.q:
            self.q.popleft()()


def build_kernel(tc: "tile.TileContext", outs, ins):
    nc = tc.nc
    x_d = ins["x"]
    out_d = outs["out"]

    ctx = contextlib.ExitStack()
    with ctx:
        const = ctx.enter_context(tc.tile_pool(name="const", bufs=1))
        wpool = ctx.enter_context(tc.tile_pool(name="wts", bufs=1))
        iop = ctx.enter_context(tc.tile_pool(name="iop", bufs=3))
        work = ctx.enter_context(tc.tile_pool(name="work", bufs=1))
        probs_pool = ctx.enter_context(tc.tile_pool(name="probs", bufs=1))
        small = ctx.enter_context(tc.tile_pool(name="small", bufs=2))
        # PSUM budget (8 banks): sq 2x2 + pv 2x1 + mm 2x1 = 8
        psum = ctx.enter_context(tc.tile_pool(name="psum", bufs=1, space="PSUM"))

        # ---- one-time constants ----
        ident = const.tile([P, P], BF16)
        make_identity(nc, ident)

        ones_row = const.tile([1, P], F32)
        nc.vector.memset(ones_row, 1.0)
        ones_r = const.tile([1, P], F32R)
        nc.vector.tensor_copy(ones_r, ones_row)

        # +ln(16) folded into exp keeps probs away from the denormal range;
        # the factor cancels between P@V numerator and denominators
        ln16 = const.tile([P, 1], F32)
        nc.vector.memset(ln16, float(np.log(16.0)))

        # q/k biases laid out per-partition: [p, oc] = b[oc*128 + p]
        bq = const.tile([P, CC], F32)
        bk = const.tile([P, CC], F32)
        with nc.allow_non_contiguous_dma(reason="tiny bias load"):
            nc.sync.dma_start(bq, ins["wq_b"].rearrange("(oc p) -> p oc", p=P))
            nc.sync.dma_start(bk, ins["wk_b"].rearrange("(oc p) -> p oc", p=P))

        # ---- x / weight streaming ----
        def x_dma(b, tt):
            xf = iop.tile([P, D], F32, tag="nat_f", name="xf", bufs=4)
            nc.sync.dma_start(xf, x_d[b, tt * P : (tt + 1) * P, :])
            return xf

        xT = work.tile([P, CC, S], BF16, tag="xT", name="xT")

        def _cast(dst, src, eng):
            if eng == "scalar":
                nc.scalar.copy(dst, src)
            else:
                nc.vector.tensor_copy(dst, src)

        def xT_unit(xf, tt, ptr_tag="mm", cast_eng="vector"):
            def emit():
                xb = iop.tile([P, D], BF16, tag="nat_b", name="xb", bufs=4)
                _cast(xb, xf, cast_eng)
                if ptr_tag == "sq":
                    ptr = psum.tile(
                        [P, CC, P], BF16, tag="sq", bufs=2, name="xptr"
                    )
                else:
                    ptr = psum.tile(
                        [P, CC, P], BF16, tag="mm", bufs=2, name="xptr2"
                    )
                for cc in range(CC):
                    nc.tensor.transpose(
                        ptr[:, cc], xb[:, cc * P : (cc + 1) * P], ident
                    )
                nc.vector.tensor_copy(xT[:, :, tt * P : (tt + 1) * P], ptr)

            return emit

        wT = {}
        for wname in ("wq_w", "wk_w", "wv_w", "wo_w"):
            wT[wname] = wpool.tile([P, CC, D], BF16, name=f"{wname}_T")

        def w_dma(name, oc, eng=None):
            wf = iop.tile([P, D], F32, tag="nat_f", name="wf", bufs=4)
            (eng or nc.gpsimd).dma_start(wf, ins[name][oc * P : (oc + 1) * P, :])
            return wf

        def wT_unit(name, wf, oc, ptr_tag="mm", cast_eng="vector"):
            def emit():
                wb = iop.tile([P, D], BF16, tag="nat_b", name="wb", bufs=4)
                _cast(wb, wf, cast_eng)
                if ptr_tag == "sq":
                    ptr = psum.tile(
                        [P, CC, P], BF16, tag="sq", bufs=2, name="wptr"
                    )
                else:
                    ptr = psum.tile(
                        [P, CC, P], BF16, tag="mm", bufs=2, name="wptr2"
                    )
                for cc in range(CC):
                    nc.tensor.transpose(
                        ptr[:, cc], wb[:, cc * P : (cc + 1) * P], ident
                    )
                nc.vector.tensor_copy(
                    wT[name][:, :, oc * P : (oc + 1) * P], ptr
                )

            return emit

        # v/out biases broadcast along partitions: [128, 768] via ones-matmul
        bias_bc = {}

        def build_bias_bc():
            for name in ("wv_b", "wo_b"):
                brow_f = iop.tile([1, D], F32, name=f"{name}_rowf", tag="nat_f", bufs=4)
                nc.sync.dma_start(brow_f, ins[name][None, :])
                brow = iop.tile([1, D], F32R, name=f"{name}_row", tag="brow", bufs=1)
                nc.vector.tensor_copy(brow, brow_f)
                bc = const.tile([P, D], BF16, name=f"{name}_bc")
                for n0 in range(0, D, TN):
                    nsz = min(TN, D - n0)
                    pb = psum.tile([P, TN], F32, tag="mm", bufs=2, name="pbias")
                    nc.tensor.matmul(
                        pb[:, :nsz], ones_r, brow[:, n0 : n0 + nsz],
                        start=True, stop=True,
                    )
                    nc.vector.tensor_copy(bc[:, n0 : n0 + nsz], pb[:, :nsz])
                bias_bc[name] = bc

        # ---- per-batch persistent tiles ----
        qT = work.tile([P, CC, S], BF16, tag="qT", name="qT")
        kT = work.tile([P, CC, S], BF16, tag="kT", name="kT")
        v_aug = work.tile([P, TT, H, HD + 1], BF16, tag="v_aug", name="v_aug")
        nc.vector.memset(v_aug[:, :, :, HD : HD + 1], 1.0)
        out_bf = work.tile([P, TT, D], BF16, tag="out_bf", name="out_bf")

        def proj_qk_unit(dst, wname, bap, oc, nt):
            def emit():
                wt = wT[wname]
                pq = psum.tile([P, TN], F32, tag="mm", bufs=2, name="pq")
                for cc in range(CC):
                    nc.tensor.matmul(
                        pq,
                        wt[:, cc, oc * P : (oc + 1) * P],
                        xT[:, cc, nt * TN : (nt + 1) * TN],
                        start=(cc == 0),
                        stop=(cc == CC - 1),
                    )
                nc.vector.tensor_tensor(
                    dst[:, oc, nt * TN : (nt + 1) * TN],
                    pq,
                    bap[:, oc : oc + 1].to_broadcast((P, TN)),
                    ALU.add,
                )

            return emit

        def chunk_units(oc):
            return [
                proj_qk_unit(dst, wname, bap, oc, nt)
                for dst, wname, bap in ((qT, "wq_w", bq), (kT, "wk_w", bk))
                for nt in range(NT)
            ]

        QTN = 256  # v sweep width: 4 heads per sweep

        def v_unit(sw, mt):
            def emit():
                n0 = sw * QTN
                wv = wT["wv_w"]
                pv = psum.tile([P, TN], F32, tag="mm", bufs=2, name="pv")
                for cc in range(CC):
                    nc.tensor.matmul(
                        pv[:, :QTN],
                        xT[:, cc, mt * P : (mt + 1) * P],
                        wv[:, cc, n0 : n0 + QTN],
                        start=(cc == 0),
                        stop=(cc == CC - 1),
                    )
                h0 = n0 // HD
                nh = QTN // HD
                nc.vector.tensor_tensor(
                    v_aug[:, mt, h0 : h0 + nh, 0:HD],
                    pv[:, :QTN].rearrange("p (h d) -> p h d", d=HD),
                    bias_bc["wv_b"][:, n0 : n0 + QTN].rearrange(
                        "p (h d) -> p h d", d=HD
                    ),
                    ALU.add,
                )

            return emit

        def v_units(sw):
            return [v_unit(sw, mt) for mt in range(TT)]

        # ---- output projection, split cc 0-3 (A) and cc 4-5 (B) ----
        def phaseA_unit(attn_T, mt, n0, nsz):
            def emit():
                wo = wT["wo_w"]
                pf = psum.tile([P, TN], F32, tag="mm", bufs=2, name="pfA")
                for cc in range(4):
                    nc.tensor.matmul(
                        pf[:, :nsz],
                        attn_T[:, cc, mt * P : (mt + 1) * P],
                        wo[:, cc, n0 : n0 + nsz],
                        start=(cc == 0),
                        stop=(cc == 3),
                    )
                nc.vector.tensor_tensor(
                    out_bf[:, mt, n0 : n0 + nsz],
                    pf[:, :nsz],
                    bias_bc["wo_b"][:, n0 : n0 + nsz],
                    ALU.add,
                )

            return emit

        def phaseA_units(attn_T):
            return [
                phaseA_unit(attn_T, mt, n0, min(TN, D - n0))
                for mt in range(TT)
                for n0 in range(0, D, TN)
            ]

        def phaseB_unit(b, attn_T, mt):
            def emit():
                wo = wT["wo_w"]
                out_sb = iop.tile([P, D], F32, tag="out_sb", bufs=2, name="out_sb")
                for n0 in range(0, D, TN):
                    nsz = min(TN, D - n0)
                    pf = psum.tile([P, TN], F32, tag="mm", bufs=2, name="pfB")
                    for cc in (4, 5):
                        nc.tensor.matmul(
                            pf[:, :nsz],
                            attn_T[:, cc, mt * P : (mt + 1) * P],
                            wo[:, cc, n0 : n0 + nsz],
                            start=(cc == 4),
                            stop=(cc == 5),
                        )
                    nc.vector.tensor_tensor(
                        out_sb[:, n0 : n0 + nsz],
                        pf[:, :nsz],
                        out_bf[:, mt, n0 : n0 + nsz],
                        ALU.add,
                    )
                nc.sync.dma_start(out_d[b, mt * P : (mt + 1) * P, :], out_sb)

            return emit

        def phaseB_units(b, attn_T):
            return [phaseB_unit(b, attn_T, mt) for mt in range(TT)]

        def out_full_unit(b, attn_T, mt):
            # single-pass output projection (all 6 chunks); used for b0
            # whose whole out_proj runs as filler inside b1's pairs
            def emit():
                wo = wT["wo_w"]
                out_sb = iop.tile([P, D], F32, tag="out_sb", bufs=2, name="out_sbf")
                for n0 in range(0, D, TN):
                    nsz = min(TN, D - n0)
                    pf = psum.tile([P, TN], F32, tag="mm", bufs=2, name="pff")
                    for cc in range(CC):
                        nc.tensor.matmul(
                            pf[:, :nsz],
                            attn_T[:, cc, mt * P : (mt + 1) * P],
                            wo[:, cc, n0 : n0 + nsz],
                            start=(cc == 0),
                            stop=(cc == CC - 1),
                        )
                    nc.vector.tensor_tensor(
                        out_sb[:, n0 : n0 + nsz],
                        pf[:, :nsz],
                        bias_bc["wo_b"][:, n0 : n0 + nsz],
                        ALU.add,
                    )
                nc.sync.dma_start(out_d[b, mt * P : (mt + 1) * P, :], out_sb)

            return emit

        # ---- per-pair normalization (per token-half ic), PE-free:
        # recip rows live in two single-row tiles (partition 0 each),
        # GpSimd broadcasts them across the head's 64 partitions, DVE
        # multiplies attn in place.
        def norm_ic(attn_T, pair, rsAB, ic):
            # broadcast [1, 2, TN] (partition 0) to all 128 partitions,
            # then per-half multiplies pick the right recip row
            icsl = slice(ic * TN, (ic + 1) * TN)
            rb = small.tile([P, 2, TN], F32, tag="rb", bufs=2)
            nc.gpsimd.partition_broadcast(rb, rsAB[0:1, :, icsl])
            sl = attn_T[:, pair, icsl]
            nc.vector.tensor_tensor(
                sl[0:HD, :], sl[0:HD, :], rb[0:HD, 0, :], ALU.mult
            )
            nc.vector.tensor_tensor(
                sl[HD:P, :], sl[HD:P, :], rb[HD:P, 1, :], ALU.mult
            )

        def make_norm(attn_T, pair, rsAB):
            def emit():
                for ic in range(NT):
                    norm_ic(attn_T, pair, rsAB, ic)

            return emit

        # ---- attention pair ----
        # Software-pipelined: QK(k) is paced by its score bank (freed by
        # EXP k-2); PV steps for EXP k-2 ride in the same slot, so the
        # PE stream self-paces with the exp stream with ~0.2us/slot to
        # spare.  External fillers (fs=small ~0.6us, fb=big ~1.4us) slot
        # into the ic1 drain phase and every few triples.
        def emit_pair(attn_T, pair, fs, fb, pre=None, post_ic0=None):
            # pre (prev pair's normalization) is PE-free; run it before
            # this pair re-allocates the single-buffered rsAB/stage ring
            # slots so its reads precede the new generation
            if pre is not None:
                pre()
            probsT_ic = []
            pos = {}
            rsAB = small.tile([1, 2, S], F32, tag="rsAB", bufs=1)
            stage = small.tile([P, 2, TN], F32, tag="stage", bufs=1)

            def qk_step(s):
                ic, jt = divmod(s, JT)
                if jt == 0:
                    probsT = probs_pool.tile(
                        [P, 2, JT, TN], BF16, tag="probsT", bufs=2,
                        name="probsT",
                    )
                    probsT_ic.append(probsT)
                sq = psum.tile([P, 2, TN], F32, tag="sq", bufs=2, name="sq")
                for hi in range(2):
                    hp = hi * HD
                    nc.tensor.matmul(
                        sq[:, hi],
                        kT[hp : hp + HD, pair, jt * P : (jt + 1) * P],
                        qT[hp : hp + HD, pair, ic * TN : (ic + 1) * TN],
                        start=True,
                        stop=True,
                    )
                nc.scalar.activation(
                    probsT_ic[ic][:, :, jt, :],
                    sq,
                    AF.Exp,
                    bias=ln16,
                    scale=float(1.0 / np.sqrt(HD)),
                )

            def pv_step(ic, jt, hi):
                h = pair * 2 + hi
                if (ic, hi) not in pos:
                    pos[(ic, hi)] = psum.tile(
                        [P, TN], F32, tag="pv", bufs=2, name="po"
                    )
                nc.tensor.matmul(
                    pos[(ic, hi)][: HD + 1, :],
                    v_aug[:, jt, h, :],
                    probsT_ic[ic][:, hi, jt, :],
                    start=(jt == 0),
                    stop=(jt == JT - 1),
                )

            def pv_evac(ic, hi):
                icsl = slice(ic * TN, (ic + 1) * TN)
                po = pos[(ic, hi)]
                nc.vector.tensor_copy(
                    stage[HD : HD + 1, hi, :], po[HD : HD + 1, :]
                )
                if hi == 0:
                    nc.vector.tensor_copy(attn_T[0:HD, pair, icsl], po[:HD, :])
                else:
                    # DVE lanes can't cross partitions; bounce via DMA
                    tmp = small.tile([HD, TN], BF16, tag="odd_tmp", bufs=2)
                    nc.vector.tensor_copy(tmp, po[:HD, :])
                    nc.gpsimd.dma_start(attn_T[HD:P, pair, icsl], tmp)

            def recip_ic(ic):
                # NOTE: reciprocal_approx_fast requires base partition 0,
                # so the sums hop to rsAB first and reciprocal in place
                icsl = slice(ic * TN, (ic + 1) * TN)
                nc.gpsimd.dma_start(
                    rsAB[0:1, :, icsl], stage[HD : HD + 1, :, :]
                )
                nc.vector.reciprocal_approx_fast(
                    rsAB[0:1, :, icsl], rsAB[0:1, :, icsl]
                )

            # ic0 phase: QK s0..15 with PV(ic0) steps two EXPs behind
            qk_step(0)
            qk_step(1)
            for s in range(2, 2 * JT):
                qk_step(s)
                e = s - 2
                jt, hi = divmod(e, 2)
                pv_step(0, jt, hi)
                if s % 3 == 1:
                    fs.run(1)
            # drain phase: last PV(ic0) steps, evac, recip; then PV(ic1)
            pv_step(0, 7, 0)
            pv_step(0, 7, 1)
            pv_evac(0, 0)
            pv_evac(0, 1)
            recip_ic(0)
            if post_ic0 is not None:
                post_ic0(rsAB)
            for e in range(2 * JT):
                jt, hi = divmod(e, 2)
                pv_step(1, jt, hi)
                if e % 2 == 0:
                    (fb if e % 4 == 0 else fs).run(1)
            pv_evac(1, 0)
            pv_evac(1, 1)
            recip_ic(1)
            return rsAB

        # ================= emission schedule (BPC == 2) =================
        # Startup: x(b0) + first weight chunks; get to pair 0 fast.
        # DMA issue order on sync: x0-3, wq0, wk0, x4-7 (so the first
        # weight chunks arrive early enough to transpose between x tiles).
        wq0 = w_dma("wq_w", 0)
        wk0 = w_dma("wk_w", 0)
        xfs = [x_dma(0, tt) for tt in range(TT)]
        wvf = [w_dma("wv_w", oc) for oc in range(CC)]
        build_bias_bc()
        for tt in range(TT):
            xT_unit(xfs[tt], tt, "sq", cast_eng="scalar")()
        wT_unit("wq_w", wq0, 0, "sq", cast_eng="scalar")()
        wT_unit("wk_w", wk0, 0, "sq", cast_eng="scalar")()
        for u in chunk_units(0):
            u()
        for oc in range(CC):
            wT_unit("wv_w", wvf[oc], oc, "sq", cast_eng="scalar")()
        for u in v_units(0):
            u()
        # remaining weight DMAs (sync queue, in consumption order)
        wqk_rest = [
            (name, oc, w_dma(name, oc, eng=nc.sync))
            for oc in range(1, CC)
            for name in ("wq_w", "wk_w")
        ]
        wof = [w_dma("wo_w", oc, eng=nc.sync) for oc in range(CC)]
        wqkT = [wT_unit(n, f, oc) for n, oc, f in wqk_rest]
        woT = [wT_unit("wo_w", wof[oc], oc) for oc in range(CC)]

        # ---- batch 0 ----
        attn0 = work.tile([P, CC, S], BF16, tag="attn_T", bufs=2, name="attn0")
        fs0 = [Filler() for _ in range(NP)]
        fb0 = [Filler() for _ in range(NP)]
        fs0[0].add(wqkT[0:4] + v_units(1)[:4])
        fb0[0].add(chunk_units(1))
        fs0[1].add(wqkT[4:10] + v_units(1)[4:])
        fb0[1].add(chunk_units(2))
        fs0[2].add(woT + v_units(2)[:4])
        fb0[2].add(chunk_units(3))
        fs0[3].add(v_units(2)[4:])
        fb0[3].add(chunk_units(4))
        fb0[4].add(chunk_units(5))
        recips0 = []
        for pair in range(NP):
            if pair == 3:
                # next batch x: DMA now (sync queue, weights all sent);
                # transpose during pairs 4-5 once chunk5/v-sweep reads of
                # this batch's xT are emitted
                # all x(b1) transposes go to pair 5's slots: they
                # overwrite xT, which chunk5 units (fb0[4]) still read
                nxfs = [x_dma(1, tt) for tt in range(TT)]
                xTb1 = [xT_unit(nxfs[tt], tt) for tt in range(TT)]
                fs0[5].add(xTb1)
            pre = (
                make_norm(attn0, pair - 1, recips0[pair - 1])
                if pair > 0
                else None
            )
            recips0.append(
                emit_pair(attn0, pair, fs0[pair], fb0[pair], pre=pre)
            )
            fb0[pair].drain()
            fs0[pair].drain()

        # batch 1 preamble: its chunk 0 projection and the first half of
        # its v sweep 0 must precede its pair 0's QK/PV emissions; b0's
        # last norm runs after them so its recip has resolved
        vb1 = v_units(0)
        for u in chunk_units(0):
            u()
        for u in vb1[:4]:
            u()
        make_norm(attn0, NP - 1, recips0[NP - 1])()

        # ---- batch 1 ----
        attn1 = work.tile([P, CC, S], BF16, tag="attn_T", bufs=2, name="attn1")
        pa1 = phaseA_units(attn1)
        pb1 = phaseB_units(1, attn1)
        out0 = [out_full_unit(0, attn0, mt) for mt in range(TT)]
        fs1 = [Filler() for _ in range(NP)]
        fb1 = [Filler() for _ in range(NP)]
        fs1[0].add(vb1[4:])
        fb1[0].add(chunk_units(1))
        fs1[1].add(v_units(1))
        fb1[1].add(chunk_units(2) + out0[:2])
        fb1[2].add(chunk_units(3) + out0[2:5])
        fs1[3].add(v_units(2))
        fb1[3].add(chunk_units(4) + out0[5:7])
        fs1[4].add(pa1[:6])
        fb1[4].add(chunk_units(5) + out0[7:])
        fs1[5].add(pa1[6:])

        def tail_post(recip2, fs=None, fb=None):
            # after pair 5's ic0 recip: drain remaining phase-A fillers,
            # normalize chunk 5's first token half, and project+store
            # token tiles 0-3 while ic1's exp stream is still going
            fs.drain()
            fb.drain()
            norm_ic(attn1, NP - 1, recip2, 0)
            for u in pb1[:4]:
                u()

        import functools

        recips1 = []
        for pair in range(NP):
            pre = (
                make_norm(attn1, pair - 1, recips1[pair - 1])
                if pair > 0
                else None
            )
            post = (
                functools.partial(tail_post, fs=fs1[pair], fb=fb1[pair])
                if pair == NP - 1
                else None
            )
            recips1.append(
                emit_pair(attn1, pair, fs1[pair], fb1[pair], pre=pre,
                          post_ic0=post)
            )
            fb1[pair].drain()
            fs1[pair].drain()
        norm_ic(attn1, NP - 1, recips1[NP - 1], 1)
        for u in pb1[4:]:
            u()


_BUILD_LOCK = threading.Lock()
_BUILT = {}


def build():
    with _BUILD_LOCK:
        if "nc" in _BUILT:
            return _BUILT["nc"]
        nc = bacc.Bacc(
            "TRN2",
            target_bir_lowering=False,
            debug=False,
            enable_asserts=True,
            num_devices=N_CORES,
        )
        ins = {
            "x": nc.dram_tensor("x", [BPC, S, D], F32, kind="ExternalInput").ap(),
        }
        for w in ("wq_w", "wk_w", "wv_w", "wo_w"):
            ins[w] = nc.dram_tensor(w, [D, D], F32, kind="ExternalInput").ap()
        for bn in ("wq_b", "wk_b", "wv_b", "wo_b"):
            ins[bn] = nc.dram_tensor(bn, [D], F32, kind="ExternalInput").ap()
        outs = {
            "out": nc.dram_tensor(
                "out", [BPC, S, D], F32, kind="ExternalOutput"
            ).ap()
        }
        with tile.TileContext(nc) as tc:
            build_kernel(tc, outs, ins)
        nc.compile()
        _BUILT["nc"] = nc
        return nc


def make_in_maps(inputs):
    x = np.ascontiguousarray(np.asarray(inputs["x"], dtype=np.float32))
    shared = {
        k: np.ascontiguousarray(np.asarray(inputs[k], dtype=np.float32))
        for k in (
            "wq_w", "wq_b", "wk_w", "wk_b", "wv_w", "wv_b", "wo_w", "wo_b",
        )
    }
    in_maps = []
    for c in range(N_CORES):
        m = {"x": x[c * BPC : (c + 1) * BPC]}
        m.update(shared)
        in_maps.append(m)
    return in_maps


def _ensure_profile_hook():
    """Install the axon NTFF profile hook shim if the container lacks it."""
    try:
        from antenv.axon_hooks import get_axon_ntff_profile_hook  # noqa: F401

        return
    except ImportError:
        pass
    try:
        import sys
        import types

        from trn_agent_boot.trn_boot import _ntff_profile_via_ctypes

        state = {"h": None}
        mod = types.ModuleType("antenv.axon_hooks")
        mod.set_axon_ntff_profile_hook = lambda h: state.__setitem__("h", h)
        mod.get_axon_ntff_profile_hook = lambda: state["h"]
        sys.modules["antenv.axon_hooks"] = mod
        mod.set_axon_ntff_profile_hook(
            _ntff_profile_via_ctypes("/opt/axon/libaxon_pjrt.so")
        )

        import concourse.bass_utils as bu

        orig_upload = bu.upload_artifacts

        def _safe_upload(d, *a, **k):
            try:
                return orig_upload(d, *a, **k)
            except Exception:
                return str(d)

        bu.upload_artifacts = _safe_upload
    except Exception:
        pass


def run(inputs, trace=False, **kwargs):
    """Returns (full_output [B,S,D] f32, BassKernelResults)."""
    if trace:
        _ensure_profile_hook()
    nc = build()
    res = run_bass_kernel_spmd(
        nc, make_in_maps(inputs), core_ids=list(range(N_CORES)),
        trace=trace, **kwargs,
    )
    out = np.concatenate([res.results[c]["out"] for c in range(N_CORES)], axis=0)
    return out, res


def kernel(**inputs):
    try:
        out, _ = run(inputs, trace=False)
    except Exception:
        # transient device hiccups (e.g. a prior crashed session) recover
        # on retry; the graph is already built/compiled at this point
        out, _ = run(inputs, trace=False)
    return out


# revision 34
# speedup vs baseline: 1.1951x; 1.1669x over previous
"""Bass/Tile kernel for multi-head self-attention on 8 TRN2 NeuronCores.

Problem: B=16, S=1024, D=768, H=12, head_dim=64, fp32 in/out.
Strategy: data parallel over batch (2 batch items per core, no collectives).

Layout follows the proven baseline (bf16 matmul operands, fp32 accum):
  - xT [c, t] feature-major via PE transposes; weights pre-transposed to
    wT [c_in, c_out]; qT/kT feature-major [o, t]; v token-major with a
    ones column per head (v_aug) so P@V emits softmax denominators in
    its last output row; scoresT = kT_h.T @ qT_h with exp on ScalarE
    straight out of PSUM (2 banks, FD=1024, +ln16 bias).

Scheduling is rebuilt around one principle: the PE queue is in-order,
so every instruction that waits on the exp stream (QK needing a free
score bank, PV needing probsT) must have independent filler matmuls
emitted BEFORE it, or the PE idles and drops out of its p-state ramp
(2.4 -> 1.2 GHz).  Concretely:
  - QK/PV emission is interleaved with filler units pulled from a
    per-pair work list: next q/k projection chunk, v sweeps (split in
    three 4-head sweeps), next batch's x transposes, wo transpose,
    and the previous batch's output projection.
  - Normalization is per-pair: P@V's sum row is DMA'd straight from
    PSUM into a [2, S] staging tile, reciprocal'd on DVE, and the
    selector-matmul broadcast + multiply run one pair later while the
    PE is warm.  The selector is a single [2, 128] constant (head 0 of
    the pair -> rows 0-63, head 1 -> rows 64-127 of its chunk).
  - The output projection is split: cc 0-3 accumulate into bf16
    out_bf during pairs 4-5 (norm(3) is done by then); cc 4-5 + bias
    + DMA run as soon as norm(5) lands (a short tail for the last
    batch, filler inside the next batch's pair 0 otherwise).
  - ScalarE runs nothing but exp during the pair loops (weight casts
    and broadcast evacuations live on DVE), so the exp stream is
    never stalled by its own engine.
"""

import contextlib
import threading
from collections import deque

import numpy as np

import concourse.bass as bass
import concourse.tile as tile
from concourse import bacc, mybir
from concourse.bass_utils import run_bass_kernel_spmd
from concourse.masks import make_identity

N_CORES = 8
B, S, D = 16, 1024, 768
H, HD = 12, 64
BPC = B // N_CORES  # batch items per core

P = 128
CC = D // P          # 6 feature chunks of 128
TN = 512             # matmul moving free dim
NT = S // TN         # 2 token chunks of 512
TT = S // P          # 8 token tiles of 128
JT = S // P          # 8 key tiles of 128
HPC = P // HD        # 2 heads per feature chunk
NP = H // 2          # 6 head pairs

F32 = mybir.dt.float32
BF16 = mybir.dt.bfloat16
F32R = mybir.dt.float32r

AF = mybir.ActivationFunctionType
ALU = mybir.AluOpType


class Filler:
    """FIFO of small emission closures interleaved into the PE stream."""

    def __init__(self):
        self.q = deque()

    def add(self, units):
        self.q.extend(units)

    def run(self, n=1):
        for _ in range(n):
            if self.q:
                self.q.popleft()()

    def drain(self):
        while self.q:
            self.q.popleft()()


def build_kernel(tc: "tile.TileContext", outs, ins):
    nc = tc.nc
    x_d = ins["x"]
    out_d = outs["out"]

    ctx = contextlib.ExitStack()
    with ctx:
        const = ctx.enter_context(tc.tile_pool(name="const", bufs=1))
        wpool = ctx.enter_context(tc.tile_pool(name="wts", bufs=1))
        iop = ctx.enter_context(tc.tile_pool(name="iop", bufs=3))
        work = ctx.enter_context(tc.tile_pool(name="work", bufs=1))
        probs_pool = ctx.enter_context(tc.tile_pool(name="probs", bufs=1))
        small = ctx.enter_context(tc.tile_pool(name="small", bufs=2))
        # PSUM budget (8 banks): sq 2x2 + pv 2x1 + mm 2x1 = 8
        psum = ctx.enter_context(tc.tile_pool(name="psum", bufs=1, space="PSUM"))

        # ---- one-time constants ----
        ident = const.tile([P, P], BF16)
        make_identity(nc, ident)

        ones_row = const.tile([1, P], F32)
        nc.vector.memset(ones_row, 1.0)
        ones_r = const.tile([1, P], F32R)
        nc.vector.tensor_copy(ones_r, ones_row)

        # pair-selector: col j of head-half h is 1; broadcasts the recip
        # rows across the chunk's 128 partitions via a tiny K=2 matmul
        sel2_f = iop.tile([2, P], F32, tag="nat_f", name="sel2_f", bufs=4)
        nc.sync.dma_start(sel2_f, ins["sel2"])
        sel2 = const.tile([2, P], BF16)
        nc.vector.tensor_copy(sel2, sel2_f)

        # +ln(16) folded into exp keeps probs away from the denormal range;
        # the factor cancels between P@V numerator and denominators
        ln16 = const.tile([P, 1], F32)
        nc.vector.memset(ln16, float(np.log(16.0)))

        # q/k biases laid out per-partition: [p, oc] = b[oc*128 + p]
        bq = const.tile([P, CC], F32)
        bk = const.tile([P, CC], F32)
        with nc.allow_non_contiguous_dma(reason="tiny bias load"):
            nc.sync.dma_start(bq, ins["wq_b"].rearrange("(oc p) -> p oc", p=P))
            nc.sync.dma_start(bk, ins["wk_b"].rearrange("(oc p) -> p oc", p=P))

        # ---- x / weight streaming ----
        def x_dma(b, tt):
            xf = iop.tile([P, D], F32, tag="nat_f", name="xf", bufs=4)
            nc.sync.dma_start(xf, x_d[b, tt * P : (tt + 1) * P, :])
            return xf

        xT = work.tile([P, CC, S], BF16, tag="xT", name="xT")

        def _cast(dst, src, eng):
            if eng == "scalar":
                nc.scalar.copy(dst, src)
            else:
                nc.vector.tensor_copy(dst, src)

        def xT_unit(xf, tt, ptr_tag="mm", cast_eng="vector"):
            def emit():
                xb = iop.tile([P, D], BF16, tag="nat_b", name="xb", bufs=4)
                _cast(xb, xf, cast_eng)
                if ptr_tag == "sq":
                    ptr = psum.tile(
                        [P, CC, P], BF16, tag="sq", bufs=2, name="xptr"
                    )
                else:
                    ptr = psum.tile(
                        [P, CC, P], BF16, tag="mm", bufs=2, name="xptr2"
                    )
                for cc in range(CC):
                    nc.tensor.transpose(
                        ptr[:, cc], xb[:, cc * P : (cc + 1) * P], ident
                    )
                nc.vector.tensor_copy(xT[:, :, tt * P : (tt + 1) * P], ptr)

            return emit

        wT = {}
        for wname in ("wq_w", "wk_w", "wv_w", "wo_w"):
            wT[wname] = wpool.tile([P, CC, D], BF16, name=f"{wname}_T")

        def w_dma(name, oc, eng=None):
            wf = iop.tile([P, D], F32, tag="nat_f", name="wf", bufs=4)
            (eng or nc.gpsimd).dma_start(wf, ins[name][oc * P : (oc + 1) * P, :])
            return wf

        def wT_unit(name, wf, oc, ptr_tag="mm", cast_eng="vector"):
            def emit():
                wb = iop.tile([P, D], BF16, tag="nat_b", name="wb", bufs=4)
                _cast(wb, wf, cast_eng)
                if ptr_tag == "sq":
                    ptr = psum.tile(
                        [P, CC, P], BF16, tag="sq", bufs=2, name="wptr"
                    )
                else:
                    ptr = psum.tile(
                        [P, CC, P], BF16, tag="mm", bufs=2, name="wptr2"
                    )
                for cc in range(CC):
                    nc.tensor.transpose(
                        ptr[:, cc], wb[:, cc * P : (cc + 1) * P], ident
                    )
                nc.vector.tensor_copy(
                    wT[name][:, :, oc * P : (oc + 1) * P], ptr
                )

            return emit

        # v/out biases broadcast along partitions: [128, 768] via ones-matmul
        bias_bc = {}

        def build_bias_bc():
            for name in ("wv_b", "wo_b"):
                brow_f = iop.tile([1, D], F32, name=f"{name}_rowf", tag="nat_f", bufs=4)
                nc.sync.dma_start(brow_f, ins[name][None, :])
                brow = iop.tile([1, D], F32R, name=f"{name}_row", tag="brow", bufs=1)
                nc.vector.tensor_copy(brow, brow_f)
                bc = const.tile([P, D], BF16, name=f"{name}_bc")
                for n0 in range(0, D, TN):
                    nsz = min(TN, D - n0)
                    pb = psum.tile([P, TN], F32, tag="mm", bufs=2, name="pbias")
                    nc.tensor.matmul(
                        pb[:, :nsz], ones_r, brow[:, n0 : n0 + nsz],
                        start=True, stop=True,
                    )
                    nc.vector.tensor_copy(bc[:, n0 : n0 + nsz], pb[:, :nsz])
                bias_bc[name] = bc

        # ---- per-batch persistent tiles ----
        qT = work.tile([P, CC, S], BF16, tag="qT", name="qT")
        kT = work.tile([P, CC, S], BF16, tag="kT", name="kT")
        v_aug = work.tile([P, TT, H, HD + 1], BF16, tag="v_aug", name="v_aug")
        nc.vector.memset(v_aug[:, :, :, HD : HD + 1], 1.0)
        out_bf = work.tile([P, TT, D], BF16, tag="out_bf", name="out_bf")

        def proj_qk_unit(dst, wname, bap, oc, nt):
            def emit():
                wt = wT[wname]
                pq = psum.tile([P, TN], F32, tag="mm", bufs=2, name="pq")
                for cc in range(CC):
                    nc.tensor.matmul(
                        pq,
                        wt[:, cc, oc * P : (oc + 1) * P],
                        xT[:, cc, nt * TN : (nt + 1) * TN],
                        start=(cc == 0),
                        stop=(cc == CC - 1),
                    )
                nc.vector.tensor_tensor(
                    dst[:, oc, nt * TN : (nt + 1) * TN],
                    pq,
                    bap[:, oc : oc + 1].to_broadcast((P, TN)),
                    ALU.add,
                )

            return emit

        def chunk_units(oc):
            return [
                proj_qk_unit(dst, wname, bap, oc, nt)
                for dst, wname, bap in ((qT, "wq_w", bq), (kT, "wk_w", bk))
                for nt in range(NT)
            ]

        QTN = 256  # v sweep width: 4 heads per sweep

        def v_unit(sw, mt):
            def emit():
                n0 = sw * QTN
                wv = wT["wv_w"]
                pv = psum.tile([P, TN], F32, tag="mm", bufs=2, name="pv")
                for cc in range(CC):
                    nc.tensor.matmul(
                        pv[:, :QTN],
                        xT[:, cc, mt * P : (mt + 1) * P],
                        wv[:, cc, n0 : n0 + QTN],
                        start=(cc == 0),
                        stop=(cc == CC - 1),
                    )
                h0 = n0 // HD
                nh = QTN // HD
                nc.vector.tensor_tensor(
                    v_aug[:, mt, h0 : h0 + nh, 0:HD],
                    pv[:, :QTN].rearrange("p (h d) -> p h d", d=HD),
                    bias_bc["wv_b"][:, n0 : n0 + QTN].rearrange(
                        "p (h d) -> p h d", d=HD
                    ),
                    ALU.add,
                )

            return emit

        def v_units(sw):
            return [v_unit(sw, mt) for mt in range(TT)]

        # ---- output projection, split cc 0-3 (A) and cc 4-5 (B) ----
        def phaseA_unit(attn_T, mt, n0, nsz):
            def emit():
                wo = wT["wo_w"]
                pf = psum.tile([P, TN], F32, tag="mm", bufs=2, name="pfA")
                for cc in range(4):
                    nc.tensor.matmul(
                        pf[:, :nsz],
                        attn_T[:, cc, mt * P : (mt + 1) * P],
                        wo[:, cc, n0 : n0 + nsz],
                        start=(cc == 0),
                        stop=(cc == 3),
                    )
                nc.vector.tensor_tensor(
                    out_bf[:, mt, n0 : n0 + nsz],
                    pf[:, :nsz],
                    bias_bc["wo_b"][:, n0 : n0 + nsz],
                    ALU.add,
                )

            return emit

        def phaseA_units(attn_T):
            return [
                phaseA_unit(attn_T, mt, n0, min(TN, D - n0))
                for mt in range(TT)
                for n0 in range(0, D, TN)
            ]

        def phaseB_unit(b, attn_T, mt):
            def emit():
                wo = wT["wo_w"]
                out_sb = iop.tile([P, D], F32, tag="out_sb", bufs=2, name="out_sb")
                for n0 in range(0, D, TN):
                    nsz = min(TN, D - n0)
                    pf = psum.tile([P, TN], F32, tag="mm", bufs=2, name="pfB")
                    for cc in (4, 5):
                        nc.tensor.matmul(
                            pf[:, :nsz],
                            attn_T[:, cc, mt * P : (mt + 1) * P],
                            wo[:, cc, n0 : n0 + nsz],
                            start=(cc == 4),
                            stop=(cc == 5),
                        )
                    nc.vector.tensor_tensor(
                        out_sb[:, n0 : n0 + nsz],
                        pf[:, :nsz],
                        out_bf[:, mt, n0 : n0 + nsz],
                        ALU.add,
                    )
                nc.sync.dma_start(out_d[b, mt * P : (mt + 1) * P, :], out_sb)

            return emit

        def phaseB_units(b, attn_T):
            return [phaseB_unit(b, attn_T, mt) for mt in range(TT)]

        def out_full_unit(b, attn_T, mt):
            # single-pass output projection (all 6 chunks); used for b0
            # whose whole out_proj runs as filler inside b1's pairs
            def emit():
                wo = wT["wo_w"]
                out_sb = iop.tile([P, D], F32, tag="out_sb", bufs=2, name="out_sbf")
                for n0 in range(0, D, TN):
                    nsz = min(TN, D - n0)
                    pf = psum.tile([P, TN], F32, tag="mm", bufs=2, name="pff")
                    for cc in range(CC):
                        nc.tensor.matmul(
                            pf[:, :nsz],
                            attn_T[:, cc, mt * P : (mt + 1) * P],
                            wo[:, cc, n0 : n0 + nsz],
                            start=(cc == 0),
                            stop=(cc == CC - 1),
                        )
                    nc.vector.tensor_tensor(
                        out_sb[:, n0 : n0 + nsz],
                        pf[:, :nsz],
                        bias_bc["wo_b"][:, n0 : n0 + nsz],
                        ALU.add,
                    )
                nc.sync.dma_start(out_d[b, mt * P : (mt + 1) * P, :], out_sb)

            return emit

        # ---- per-pair normalization (per token-half ic), PE-free:
        # recip rows live in two single-row tiles (partition 0 each),
        # GpSimd broadcasts them across the head's 64 partitions, DVE
        # multiplies attn in place.
        def norm_ic(attn_T, pair, recip2, ic):
            # broadcast the two f32r recip rows across 128 partitions on
            # the PE; the in-place multiply reads the result straight
            # from PSUM (no staging copy)
            icsl = slice(ic * TN, (ic + 1) * TN)
            pb = psum.tile([P, TN], F32, tag="mm", bufs=2, name="pb")
            nc.tensor.matmul(
                pb, sel2, recip2[:, icsl], start=True, stop=True
            )
            sl = attn_T[:, pair, icsl]
            nc.vector.tensor_tensor(sl, sl, pb, ALU.mult)

        def make_norm(attn_T, pair, recip2):
            def emit():
                for ic in range(NT):
                    norm_ic(attn_T, pair, recip2, ic)

            return emit

        # ---- attention pair ----
        # Software-pipelined: QK(k) is paced by its score bank (freed by
        # EXP k-2); PV steps for EXP k-2 ride in the same slot, so the
        # PE stream self-paces with the exp stream with ~0.2us/slot to
        # spare.  External fillers (fs=small ~0.6us, fb=big ~1.4us) slot
        # into the ic1 drain phase and every few triples.
        def emit_pair(attn_T, pair, fs, fb, pre=None, post_ic0=None):
            # pre (prev pair's normalization) is PE-free; run it before
            # this pair re-allocates the single-buffered rsAB/stage ring
            # slots so its reads precede the new generation
            # prev pair's norm, split per token-half around two boundary
            # fillers (the fillers may read ic0-normalized attn only)
            if pre is not None:
                norm_ic(attn_T, pre[0], pre[1], 0)
            fs.run(2)
            if pre is not None:
                norm_ic(attn_T, pre[0], pre[1], 1)
            probsT_ic = []
            pos = {}
            rsum2 = small.tile([2, S], F32, tag="rsum2", bufs=1)
            recip2 = small.tile([2, S], BF16, tag="recip2", bufs=1)
            stage = small.tile([P, 2, TN], F32, tag="stage", bufs=1)

            def qk_step(s):
                ic, jt = divmod(s, JT)
                if jt == 0:
                    probsT = probs_pool.tile(
                        [P, 2, JT, TN], BF16, tag="probsT", bufs=2,
                        name="probsT",
                    )
                    probsT_ic.append(probsT)
                sq = psum.tile([P, 2, TN], F32, tag="sq", bufs=2, name="sq")
                for hi in range(2):
                    hp = hi * HD
                    nc.tensor.matmul(
                        sq[:, hi],
                        kT[hp : hp + HD, pair, jt * P : (jt + 1) * P],
                        qT[hp : hp + HD, pair, ic * TN : (ic + 1) * TN],
                        start=True,
                        stop=True,
                    )
                nc.scalar.activation(
                    probsT_ic[ic][:, :, jt, :],
                    sq,
                    AF.Exp,
                    bias=ln16,
                    scale=float(1.0 / np.sqrt(HD)),
                )

            def pv_step(ic, jt, hi):
                h = pair * 2 + hi
                if (ic, hi) not in pos:
                    pos[(ic, hi)] = psum.tile(
                        [P, TN], F32, tag="pv", bufs=2, name="po"
                    )
                nc.tensor.matmul(
                    pos[(ic, hi)][: HD + 1, :],
                    v_aug[:, jt, h, :],
                    probsT_ic[ic][:, hi, jt, :],
                    start=(jt == 0),
                    stop=(jt == JT - 1),
                )

            def pv_evac(ic, hi):
                icsl = slice(ic * TN, (ic + 1) * TN)
                po = pos[(ic, hi)]
                nc.vector.tensor_copy(
                    stage[HD : HD + 1, hi, :], po[HD : HD + 1, :]
                )
                if hi == 0:
                    nc.vector.tensor_copy(attn_T[0:HD, pair, icsl], po[:HD, :])
                else:
                    # DVE lanes can't cross partitions; bounce via DMA
                    tmp = small.tile([HD, TN], BF16, tag="odd_tmp", bufs=2)
                    nc.vector.tensor_copy(tmp, po[:HD, :])
                    nc.gpsimd.dma_start(attn_T[HD:P, pair, icsl], tmp)

            def recip_ic(ic):
                # NOTE: reciprocal_approx_fast requires base partition 0,
                # so the sums hop to rsum2 (rows 0-1) first
                icsl = slice(ic * TN, (ic + 1) * TN)
                for hi in range(2):
                    nc.gpsimd.dma_start(
                        rsum2[hi : hi + 1, icsl],
                        stage[HD : HD + 1, hi, :],
                    )
                nc.vector.reciprocal_approx_fast(
                    rsum2[:, icsl], rsum2[:, icsl]
                )
                nc.vector.tensor_copy(recip2[:, icsl], rsum2[:, icsl])

            # ic0 phase: QK s0..15 with PV(ic0) steps two EXPs behind
            qk_step(0)
            qk_step(1)
            for s in range(2, 2 * JT):
                qk_step(s)
                e = s - 2
                jt, hi = divmod(e, 2)
                pv_step(0, jt, hi)
                if s % 3 == 1:
                    fs.run(1)
            # drain phase: last PV(ic0) steps, evac, recip; then PV(ic1)
            pv_step(0, 7, 0)
            pv_step(0, 7, 1)
            pv_evac(0, 0)
            pv_evac(0, 1)
            recip_ic(0)
            if post_ic0 is not None:
                post_ic0(recip2)
            for e in range(2 * JT):
                jt, hi = divmod(e, 2)
                pv_step(1, jt, hi)
                if e % 2 == 0:
                    (fb if e % 4 == 0 else fs).run(1)
            pv_evac(1, 0)
            pv_evac(1, 1)
            recip_ic(1)
            return recip2

        # ================= emission schedule (BPC == 2) =================
        # Startup: x(b0) + first weight chunks; get to pair 0 fast.
        # DMA issue order on sync: x0-3, wq0, wk0, x4-7 (so the first
        # weight chunks arrive early enough to transpose between x tiles).
        wq0 = w_dma("wq_w", 0)
        wk0 = w_dma("wk_w", 0)
        xfs = [x_dma(0, tt) for tt in range(TT)]
        wvf = [w_dma("wv_w", oc) for oc in range(CC)]
        build_bias_bc()
        for tt in range(TT):
            xT_unit(xfs[tt], tt, "sq", cast_eng="scalar")()
        wT_unit("wq_w", wq0, 0, "sq", cast_eng="scalar")()
        wT_unit("wk_w", wk0, 0, "sq", cast_eng="scalar")()
        for u in chunk_units(0):
            u()
        for oc in range(CC):
            wT_unit("wv_w", wvf[oc], oc, "sq", cast_eng="scalar")()
        for u in v_units(0):
            u()
        # remaining weight DMAs (sync queue, in consumption order)
        wqk_rest = [
            (name, oc, w_dma(name, oc, eng=nc.sync))
            for oc in range(1, CC)
            for name in ("wq_w", "wk_w")
        ]
        wof = [w_dma("wo_w", oc, eng=nc.sync) for oc in range(CC)]
        wqkT = [wT_unit(n, f, oc) for n, oc, f in wqk_rest]
        woT = [wT_unit("wo_w", wof[oc], oc) for oc in range(CC)]

        # ---- batch 0 ----
        attn0 = work.tile([P, CC, S], BF16, tag="attn_T", bufs=2, name="attn0")
        fs0 = [Filler() for _ in range(NP)]
        fb0 = [Filler() for _ in range(NP)]
        fs0[0].add(wqkT[0:4] + v_units(1)[:4])
        fb0[0].add(chunk_units(1))
        fs0[1].add(wqkT[4:10] + v_units(1)[4:])
        fb0[1].add(chunk_units(2))
        fs0[2].add(woT + v_units(2)[:4])
        fb0[2].add(chunk_units(3))
        fs0[3].add(v_units(2)[4:])
        fb0[3].add(chunk_units(4))
        fb0[4].add(chunk_units(5))
        recips0 = []
        for pair in range(NP):
            if pair == 3:
                # next batch x: DMA now (sync queue, weights all sent);
                # transpose during pairs 4-5 once chunk5/v-sweep reads of
                # this batch's xT are emitted
                # all x(b1) transposes go to pair 5's slots: they
                # overwrite xT, which chunk5 units (fb0[4]) still read
                nxfs = [x_dma(1, tt) for tt in range(TT)]
                xTb1 = [xT_unit(nxfs[tt], tt) for tt in range(TT)]
                fs0[5].add(xTb1)
            pre = (pair - 1, recips0[pair - 1]) if pair > 0 else None
            recips0.append(
                emit_pair(attn0, pair, fs0[pair], fb0[pair], pre=pre)
            )
            fb0[pair].drain()
            fs0[pair].drain()

        # batch 1 preamble: its chunk 0 projection and the first half of
        # its v sweep 0 must precede its pair 0's QK/PV emissions; b0's
        # last norm runs after them so its recip has resolved
        vb1 = v_units(0)
        for u in chunk_units(0):
            u()
        for u in vb1[:4]:
            u()
        make_norm(attn0, NP - 1, recips0[NP - 1])()

        # ---- batch 1 ----
        attn1 = work.tile([P, CC, S], BF16, tag="attn_T", bufs=2, name="attn1")
        pa1 = phaseA_units(attn1)
        pb1 = phaseB_units(1, attn1)
        out0 = [out_full_unit(0, attn0, mt) for mt in range(TT)]
        fs1 = [Filler() for _ in range(NP)]
        fb1 = [Filler() for _ in range(NP)]
        fs1[0].add(vb1[4:])
        fb1[0].add(chunk_units(1))
        fs1[1].add(v_units(1))
        fb1[1].add(chunk_units(2) + out0[:2])
        fb1[2].add(chunk_units(3) + out0[2:5])
        fs1[3].add(v_units(2))
        fb1[3].add(chunk_units(4) + out0[5:7])
        fs1[4].add(pa1[:6])
        fb1[4].add(chunk_units(5) + out0[7:])
        fs1[5].add(pa1[6:])

        def tail_post(recip2, fs=None, fb=None):
            # after pair 5's ic0 recip: drain remaining phase-A fillers,
            # normalize chunk 5's first token half, and project+store
            # token tiles 0-3 while ic1's exp stream is still going
            fs.drain()
            fb.drain()
            norm_ic(attn1, NP - 1, recip2, 0)
            for u in pb1[:4]:
                u()

        import functools

        recips1 = []
        for pair in range(NP):
            pre = (pair - 1, recips1[pair - 1]) if pair > 0 else None
            post = (
                functools.partial(tail_post, fs=fs1[pair], fb=fb1[pair])
                if pair == NP - 1
                else None
            )
            recips1.append(
                emit_pair(attn1, pair, fs1[pair], fb1[pair], pre=pre,
                          post_ic0=post)
            )
            fb1[pair].drain()
            fs1[pair].drain()
        norm_ic(attn1, NP - 1, recips1[NP - 1], 1)
        for u in pb1[4:]:
            u()


_BUILD_LOCK = threading.Lock()
_BUILT = {}


def build():
    with _BUILD_LOCK:
        if "nc" in _BUILT:
            return _BUILT["nc"]
        nc = bacc.Bacc(
            "TRN2",
            target_bir_lowering=False,
            debug=False,
            enable_asserts=True,
            num_devices=N_CORES,
        )
        ins = {
            "x": nc.dram_tensor("x", [BPC, S, D], F32, kind="ExternalInput").ap(),
            "sel2": nc.dram_tensor(
                "sel2", [2, P], F32, kind="ExternalInput"
            ).ap(),
        }
        for w in ("wq_w", "wk_w", "wv_w", "wo_w"):
            ins[w] = nc.dram_tensor(w, [D, D], F32, kind="ExternalInput").ap()
        for bn in ("wq_b", "wk_b", "wv_b", "wo_b"):
            ins[bn] = nc.dram_tensor(bn, [D], F32, kind="ExternalInput").ap()
        outs = {
            "out": nc.dram_tensor(
                "out", [BPC, S, D], F32, kind="ExternalOutput"
            ).ap()
        }
        with tile.TileContext(nc) as tc:
            build_kernel(tc, outs, ins)
        nc.compile()
        _BUILT["nc"] = nc
        return nc


def make_in_maps(inputs):
    x = np.ascontiguousarray(np.asarray(inputs["x"], dtype=np.float32))
    shared = {
        k: np.ascontiguousarray(np.asarray(inputs[k], dtype=np.float32))
        for k in (
            "wq_w", "wq_b", "wk_w", "wk_b", "wv_w", "wv_b", "wo_w", "wo_b",
        )
    }
    sel2 = np.zeros((2, P), np.float32)
    sel2[0, :HD] = 1.0
    sel2[1, HD:] = 1.0
    in_maps = []
    for c in range(N_CORES):
        m = {"x": x[c * BPC : (c + 1) * BPC], "sel2": sel2}
        m.update(shared)
        in_maps.append(m)
    return in_maps


def _ensure_profile_hook():
    """Install the axon NTFF profile hook shim if the container lacks it."""
    try:
        from antenv.axon_hooks import get_axon_ntff_profile_hook  # noqa: F401

        return
    except ImportError:
        pass
    try:
        import sys
        import types

        from trn_agent_boot.trn_boot import _ntff_profile_via_ctypes

        state = {"h": None}
        mod = types.ModuleType("antenv.axon_hooks")
        mod.set_axon_ntff_profile_hook = lambda h: state.__setitem__("h", h)
        mod.get_axon_ntff_profile_hook = lambda: state["h"]
        sys.modules["antenv.axon_hooks"] = mod
        mod.set_axon_ntff_profile_hook(
            _ntff_profile_via_ctypes("/opt/axon/libaxon_pjrt.so")
        )

        import concourse.bass_utils as bu

        orig_upload = bu.upload_artifacts

        def _safe_upload(d, *a, **k):
            try:
                return orig_upload(d, *a, **k)
            except Exception:
                return str(d)

        bu.upload_artifacts = _safe_upload
    except Exception:
        pass


def run(inputs, trace=False, **kwargs):
    """Returns (full_output [B,S,D] f32, BassKernelResults)."""
    if trace:
        _ensure_profile_hook()
    nc = build()
    res = run_bass_kernel_spmd(
        nc, make_in_maps(inputs), core_ids=list(range(N_CORES)),
        trace=trace, **kwargs,
    )
    out = np.concatenate([res.results[c]["out"] for c in range(N_CORES)], axis=0)
    return out, res


def kernel(**inputs):
    try:
        out, _ = run(inputs, trace=False)
    except Exception:
        # transient device hiccups (e.g. a prior crashed session) recover
        # on retry; the graph is already built/compiled at this point
        out, _ = run(inputs, trace=False)
    return out
